# revision 6
# baseline (speedup 1.0000x reference)
"""BigramHash embedding lookup kernel for 8 Trainium2 NeuronCores.

Strategy (row-sharded table, host-side all-to-all since we receive full inputs):
  - Host computes bucket ids h = (prev_id * MULT + id) % NUM_BUCKETS, routes
    each token to the core owning its table shard (SHARD = 250001 rows), and
    sorts by local row id (HBM locality + windowed int16 gather indices).
  - Real tokens are spread evenly over the padded cap slots (pads forward-
    fill the previous id) so position-aligned gather chunks cover the same
    id quantiles on every core — the SPMD program bakes ONE window base per
    chunk, taken as the cross-core min of the chunk's first id (exact,
    data-derived; cache keyed on the bases tuple).
  - Primary path: gpsimd.dma_gather (SWDGE custom op) pulls NT=384 tokens
    per call — the ~1us SWDGE cost is per CALL, so 11 calls replace the 33
    serial indirect DMAs that made the old gather stream ~46us. Indices are
    int16 relative to the chunk's 32768-row window; table rows stored bf16
    padded to 128 cols (256B, the dma_gather granularity).
  - Per 128-token block: PE transpose (64 cols) into a shared [128,128]
    PSUM tile (two blocks -> two PE row groups so their projection matmuls
    stream concurrently), copy to SBUF, 2 matmuls into a 2-bank PSUM tile
    [128,1024] f32, ONE cast to bf16 (alternating vector/scalar engines),
    store. bf16 output halves the dominant HBM write vs f32 (~1e-3 rel err,
    far inside the 2e-2 gate). Host scatters slabs back to token order.
  - Fallback (any window infeasible): per-128-token-block HW indirect DMA
    (int32 ids over the whole shard) — slower but unconditionally correct.
"""

import os as _os
from contextlib import ExitStack

import ml_dtypes
import numpy as np

import concourse.bass as bass
import concourse.mybir as mybir
import concourse.tile as tile
from concourse import bacc
from concourse.bass import IndirectOffsetOnAxis
from concourse.bass_utils import run_bass_kernel_spmd

VARIANT = _os.environ.get("BIGRAM_VARIANT", "dg")  # "dg" | "ind"
NT = int(_os.environ.get("BIGRAM_NT", "384"))  # tokens per dma_gather call

NUM_BUCKETS = 2000003
HASH_DIM = 64
EPAD = 128  # padded row length (bf16) for the 256B dma_gather granularity
MODEL_DIM = 1024
HASH_MULT = 92821
N_CORES = 8
P = 128
SHARD = 250001  # ceil(NUM_BUCKETS / N_CORES); 8*250001 = 2000008 >= NUM_BUCKETS
NFREE = 512  # matmul free dim (one PSUM bank of f32)
W = 32768  # gather window rows (int16 index range)

_prog_cache: dict = {}


def _build_dg_program(K: int, bases: tuple) -> "bacc.Bacc":
    """dma_gather path: chunks of NT tokens (NT/128 blocks) per SWDGE call."""
    nts = []
    r = K * P
    while r > 0:
        nts.append(min(NT, r))
        r -= nts[-1]
    NCH = len(nts)
    assert len(bases) == NCH
    nc = bacc.Bacc(
        "TRN2",
        target_bir_lowering=False,
        debug=False,
        num_devices=N_CORES,
        dynamic_dma_scratch_size=65536,
    )
    f32 = mybir.dt.float32
    bf16 = mybir.dt.bfloat16
    ncols = (K * P) // 16
    idx_d = nc.dram_tensor(
        "idx16", [P, ncols], mybir.dt.int16, kind="ExternalInput"
    ).ap()
    tab_d = nc.dram_tensor("table", [SHARD, EPAD], bf16, kind="ExternalInput").ap()
    projT_d = nc.dram_tensor(
        "projT", [HASH_DIM, MODEL_DIM], bf16, kind="ExternalInput"
    ).ap()
    ident_d = nc.dram_tensor("ident", [P, P], bf16, kind="ExternalInput").ap()
    out_d = nc.dram_tensor("out", [P * K, MODEL_DIM], bf16, kind="ExternalOutput").ap()

    # block -> (chunk, sub-block) map; chunk col offsets into idx16
    blk_ch, blk_sub, col_off = [], [], []
    off = 0
    for ch, nt in enumerate(nts):
        col_off.append(off)
        off += nt // 16
        for s in range(nt // P):
            blk_ch.append(ch)
            blk_sub.append(s)
    assert len(blk_ch) == K

    with tile.TileContext(nc) as tc, ExitStack() as ctx:
        const_p = ctx.enter_context(tc.tile_pool(name="const", bufs=1))
        idx_p = ctx.enter_context(tc.tile_pool(name="idx", bufs=1))
        emb_p = ctx.enter_context(tc.tile_pool(name="emb", bufs=6))
        embT_p = ctx.enter_context(tc.tile_pool(name="embT", bufs=4))
        out_p = ctx.enter_context(tc.tile_pool(name="out", bufs=6))
        ps_t = ctx.enter_context(tc.tile_pool(name="ps_t", bufs=2, space="PSUM"))
        # PSUM is bank-granular: ps_t 2x1 bank + ps_mm 3x2 banks = all 8.
        ps_mm = ctx.enter_context(tc.tile_pool(name="ps_mm", bufs=3, space="PSUM"))

        # idx first: the gather stream depends only on it.
        idx_t = idx_p.tile([P, ncols], mybir.dt.int16)
        nc.sync.dma_start(out=idx_t[:], in_=idx_d[:])
        ident = const_p.tile([P, P], bf16)
        nc.sync.dma_start(out=ident[:], in_=ident_d[:])
        # projT duplicated on partitions 0-63 / 64-127 so a pair's matmuls
        # run concurrently in separate PE row groups (auto tile_position).
        projT_s = const_p.tile([P, MODEL_DIM], bf16)
        nc.sync.dma_start(out=projT_s[:HASH_DIM, :], in_=projT_d[:])
        nc.sync.dma_start(out=projT_s[HASH_DIM:, :], in_=projT_d[:])
        # PE warm-up during the DMA-wait ramp: releases the HAM clock gate
        # before the first real transpose.
        dumw = const_p.tile([P, NFREE], bf16)
        nc.vector.memset(dumw[:], 0.0)
        wps = ps_mm.tile([P, MODEL_DIM], f32, name="mm")
        for _ in range(8):
            nc.tensor.matmul(
                wps[:, :NFREE], lhsT=dumw[:, :P], rhs=dumw[:], start=True, stop=True
            )

        # If K is odd, gather the odd block's chunk first and process the
        # odd block (serial matmul chain, no row-group partner) in the ramp.
        ch_order = list(range(NCH))
        pairs = [[2 * q, 2 * q + 1] for q in range(K // 2)]
        if K % 2:
            ch_order = [NCH - 1] + ch_order[:-1]
            pairs = [[K - 1]] + pairs

        emb_tiles = {}
        for ch in ch_order:
            nt = nts[ch]
            nb = nt // P
            embp = emb_p.tile([P, nb * EPAD], bf16, name="embp")
            emb3 = embp[:].rearrange("p (c e) -> p c e", e=EPAD)
            nc.gpsimd.dma_gather(
                emb3,
                tab_d[bases[ch] : bases[ch] + W, :],
                idx_t[:, col_off[ch] : col_off[ch] + nt // 16],
                nt,
                nt,
                EPAD,
            )
            emb_tiles[ch] = embp

        cast_i = 0
        for pair in pairs:
            nb = len(pair)
            eT_ps = ps_t.tile([nb * HASH_DIM, P], bf16, name="eT_ps")
            for jj, b in enumerate(pair):
                nc.tensor.transpose(
                    eT_ps[jj * HASH_DIM : (jj + 1) * HASH_DIM, :],
                    emb_tiles[blk_ch[b]][
                        :, blk_sub[b] * EPAD : blk_sub[b] * EPAD + HASH_DIM
                    ],
                    ident[:],
                )
            eT = embT_p.tile([nb * HASH_DIM, P], bf16, name="eT")
            # Alternate the eT copies too so vector/scalar stay balanced.
            if cast_i % 2 == 0:
                nc.vector.tensor_copy(eT[:], eT_ps[:])
            else:
                nc.scalar.copy(eT[:], eT_ps[:])
            mms = [ps_mm.tile([P, MODEL_DIM], f32, name="mm") for _ in range(nb)]
            # Interleave matmuls across the two blocks so consecutive PE
            # instructions target alternating row groups and overlap.
            for h in range(MODEL_DIM // NFREE):
                for jj in range(nb):
                    nc.tensor.matmul(
                        mms[jj][:, h * NFREE : (h + 1) * NFREE],
                        lhsT=eT[jj * HASH_DIM : (jj + 1) * HASH_DIM, :],
                        rhs=projT_s[
                            jj * HASH_DIM : (jj + 1) * HASH_DIM,
                            h * NFREE : (h + 1) * NFREE,
                        ],
                        start=True,
                        stop=True,
                    )
            for jj, b in enumerate(pair):
                o_t = out_p.tile([P, MODEL_DIM], bf16, name="o_t")
                if (cast_i + jj) % 2 == 0:
                    nc.vector.tensor_copy(o_t[:], mms[jj][:])
                else:
                    nc.scalar.copy(o_t[:], mms[jj][:])
                nc.sync.dma_start(out=out_d[b * P : (b + 1) * P, :], in_=o_t[:])
            cast_i += 1
    nc.compile()
    return nc


def _build_ind_program(K: int) -> "bacc.Bacc":
    """Fallback: per-128-token-block indirect DMA gather (int32 ids).

    HW semantics allow only ONE offset per partition per call, so this path
    costs ~1.4us of gpsimd per 128 tokens — correct for any input, slow."""
    nc = bacc.Bacc(
        "TRN2",
        target_bir_lowering=False,
        debug=False,
        num_devices=N_CORES,
        dynamic_dma_scratch_size=65536,
    )
    f32 = mybir.dt.float32
    bf16 = mybir.dt.bfloat16
    idx_d = nc.dram_tensor("idx", [P, K], mybir.dt.int32, kind="ExternalInput").ap()
    tab_d = nc.dram_tensor("table", [SHARD, HASH_DIM], bf16, kind="ExternalInput").ap()
    projT_d = nc.dram_tensor(
        "projT", [HASH_DIM, MODEL_DIM], bf16, kind="ExternalInput"
    ).ap()
    ident_d = nc.dram_tensor("ident", [P, P], bf16, kind="ExternalInput").ap()
    out_d = nc.dram_tensor("out", [P * K, MODEL_DIM], bf16, kind="ExternalOutput").ap()

    with tile.TileContext(nc) as tc, ExitStack() as ctx:
        const_p = ctx.enter_context(tc.tile_pool(name="const", bufs=1))
        idx_p = ctx.enter_context(tc.tile_pool(name="idx", bufs=1))
        emb_p = ctx.enter_context(tc.tile_pool(name="emb", bufs=6))
        embT_p = ctx.enter_context(tc.tile_pool(name="embT", bufs=4))
        out_p = ctx.enter_context(tc.tile_pool(name="out", bufs=6))
        ps_t = ctx.enter_context(tc.tile_pool(name="ps_t", bufs=2, space="PSUM"))
        ps_mm = ctx.enter_context(tc.tile_pool(name="ps_mm", bufs=3, space="PSUM"))

        idx_t = idx_p.tile([P, K], mybir.dt.int32)
        nc.sync.dma_start(out=idx_t[:], in_=idx_d[:])
        ident = const_p.tile([P, P], bf16)
        nc.sync.dma_start(out=ident[:], in_=ident_d[:])
        projT_s = const_p.tile([P, MODEL_DIM], bf16)
        nc.sync.dma_start(out=projT_s[:HASH_DIM, :], in_=projT_d[:])
        nc.sync.dma_start(out=projT_s[HASH_DIM:, :], in_=projT_d[:])

        pbs = list(range(0, K, 2))
        if K % 2:
            pbs = pbs[-1:] + pbs[:-1]
        cast_i = 0
        for pb in pbs:
            nblocks = min(2, K - pb)
            embp = emb_p.tile([P, nblocks * HASH_DIM], bf16)
            for j in range(nblocks):
                nc.gpsimd.indirect_dma_start(
                    out=embp[:, j * HASH_DIM : (j + 1) * HASH_DIM],
                    out_offset=None,
                    in_=tab_d[:],
                    in_offset=IndirectOffsetOnAxis(
                        ap=idx_t[:, pb + j : pb + j + 1], axis=0
                    ),
                )
            eT_ps = ps_t.tile([nblocks * HASH_DIM, P], bf16)
            nc.tensor.transpose(eT_ps[:], embp[:], ident[:])
            eT = embT_p.tile([nblocks * HASH_DIM, P], bf16)
            if cast_i % 2 == 0:
                nc.vector.tensor_copy(eT[:], eT_ps[:])
            else:
                nc.scalar.copy(eT[:], eT_ps[:])
            mms = [ps_mm.tile([P, MODEL_DIM], f32, name="mm") for _ in range(nblocks)]
            for h in range(MODEL_DIM // NFREE):
                for jj in range(nblocks):
                    nc.tensor.matmul(
                        mms[jj][:, h * NFREE : (h + 1) * NFREE],
                        lhsT=eT[jj * HASH_DIM : (jj + 1) * HASH_DIM, :],
                        rhs=projT_s[
                            jj * HASH_DIM : (jj + 1) * HASH_DIM,
                            h * NFREE : (h + 1) * NFREE,
                        ],
                        start=True,
                        stop=True,
                    )
            for jj in range(nblocks):
                o_t = out_p.tile([P, MODEL_DIM], bf16, name="o_t")
                if (cast_i + jj) % 2 == 0:
                    nc.vector.tensor_copy(o_t[:], mms[jj][:])
                else:
                    nc.scalar.copy(o_t[:], mms[jj][:])
                nc.sync.dma_start(
                    out=out_d[(pb + jj) * P : (pb + jj + 1) * P, :], in_=o_t[:]
                )
            cast_i += 1
    nc.compile()
    return nc


def prepare(input_ids, table, proj_w):
    """Route tokens, pick program variant, build per-core in_maps."""
    B, S = input_ids.shape
    T = B * S
    ids = np.asarray(input_ids, dtype=np.int64)
    prev = np.empty_like(ids)
    prev[:, 0] = 0
    prev[:, 1:] = ids[:, :-1]
    h = ((prev * HASH_MULT + ids) % NUM_BUCKETS).reshape(-1)
    owner = h // SHARD
    local = (h - owner * SHARD).astype(np.int64)
    order = np.lexsort((local, owner))
    counts = np.bincount(owner, minlength=N_CORES).astype(np.int64)
    offsets = np.zeros(N_CORES + 1, dtype=np.int64)
    np.cumsum(counts, out=offsets[1:])
    sorted_local = local[order]

    cap = max(P, int(-(-counts.max() // P)) * P)
    K = cap // P
    nts = []
    r = cap
    while r > 0:
        nts.append(min(NT, r))
        r -= nts[-1]
    NCH = len(nts)
    cuts = np.zeros(NCH + 1, dtype=np.int64)
    np.cumsum(nts, out=cuts[1:])

    # Spread each core's real ids evenly over the cap slots; pads forward-
    # fill so the padded sequence stays sorted and chunk windows stay tight.
    pos_list, padded_list = [], []
    for c in range(N_CORES):
        loc = sorted_local[offsets[c] : offsets[c + 1]]
        n = len(loc)
        if n == 0:
            pos = np.zeros(0, dtype=np.int64)
            row = np.zeros(cap, dtype=np.int64)
        else:
            pos = (np.arange(n, dtype=np.int64) * cap) // n
            row = np.zeros(cap, dtype=np.int64)
            row[pos] = loc
            mark = np.full(cap, -1, dtype=np.int64)
            mark[pos] = np.arange(cap, dtype=np.int64)[pos]
            np.maximum.accumulate(mark, out=mark)
            row = row[np.maximum(mark, 0)]
        pos_list.append(pos)
        padded_list.append(row)
    padded_all = np.stack(padded_list)  # [N_CORES, cap]

    # Exact cross-core window bases; feasibility per chunk.
    bases = []
    ok = VARIANT == "dg"
    for ch in range(NCH):
        lo = int(padded_all[:, cuts[ch]].min())
        hi = int(padded_all[:, cuts[ch + 1] - 1].max())
        b = min(max(lo, 0), SHARD - W)
        bases.append(b)
        if hi - b > W - 1:
            ok = False
    bases = tuple(bases)

    table = np.asarray(table, dtype=np.float32)
    projT = np.ascontiguousarray(
        np.asarray(proj_w, dtype=np.float32).T.astype(ml_dtypes.bfloat16)
    )
    in_maps = []
    for c in range(N_CORES):
        lo, hi = c * SHARD, min((c + 1) * SHARD, NUM_BUCKETS)
        ncols_tab = EPAD if ok else HASH_DIM
        shard = np.zeros((SHARD, ncols_tab), dtype=ml_dtypes.bfloat16)
        shard[: hi - lo, :HASH_DIM] = table[lo:hi].astype(ml_dtypes.bfloat16)
        m = {"table": shard, "projT": projT}
        if ok:
            # idx16[p, col_off+s] = chunk token s*16+p, relative to the
            # chunk base; wrapped in 16 partitions, replicated to all 8
            # 16-partition Q7 core groups.
            rel = padded_all[c].copy()
            for ch in range(NCH):
                rel[cuts[ch] : cuts[ch + 1]] -= bases[ch]
            rel = np.maximum(rel, 0)
            cols = [
                rel[cuts[ch] : cuts[ch + 1]].reshape(-1, 16).T for ch in range(NCH)
            ]
            row16 = np.concatenate(cols, axis=1).astype(np.int16)
            m["idx16"] = np.ascontiguousarray(np.tile(row16, (P // 16, 1)))
        else:
            padded = np.zeros(cap, dtype=np.int64)
            padded[: counts[c]] = sorted_local[offsets[c] : offsets[c + 1]]
            m["idx"] = np.ascontiguousarray(padded.astype(np.int32).reshape(K, P).T)
        m["ident"] = np.eye(P, dtype=ml_dtypes.bfloat16)
        in_maps.append(m)

    key = ("dg", K, NT, bases) if ok else ("ind", K)
    nc = _prog_cache.get(key)
    if nc is None:
        nc = _build_dg_program(K, bases) if ok else _build_ind_program(K)
        _prog_cache[key] = nc
    # row_map[c]: device out row holding sorted token k of core c
    if ok:
        row_map = pos_list
    else:
        row_map = [np.arange(counts[c], dtype=np.int64) for c in range(N_CORES)]
    meta = (T, order, offsets, row_map, K)
    return nc, in_maps, meta


def kernel(input_ids: np.ndarray, table: np.ndarray, proj_w: np.ndarray) -> np.ndarray:
    B, S = input_ids.shape
    nc, in_maps, meta = prepare(input_ids, table, proj_w)
    T, order, offsets, row_map, K = meta
    res = run_bass_kernel_spmd(nc, in_maps, list(range(N_CORES)))
    flat = np.empty((T, MODEL_DIM), dtype=np.float32)
    for c in range(N_CORES):
        flat[order[offsets[c] : offsets[c + 1]]] = res.results[c]["out"][
            row_map[c]
        ].astype(np.float32)
    return flat.reshape(B, S, MODEL_DIM)


# revision 10
# speedup vs baseline: 1.0040x; 1.0040x over previous
"""BigramHash embedding lookup kernel for 8 Trainium2 NeuronCores.

Strategy (row-sharded table, host-side all-to-all since we receive full inputs):
  - Host computes bucket ids h = (prev_id * MULT + id) % NUM_BUCKETS, routes
    each token to the core owning its table shard (SHARD = 250001 rows), and
    sorts by local row id (HBM locality + windowed int16 gather indices).
  - Real tokens are spread evenly over the padded cap slots (pads forward-
    fill the previous id) so position-aligned gather chunks cover the same
    id quantiles on every core — the SPMD program bakes ONE window base per
    chunk, taken as the cross-core min of the chunk's first id (exact,
    data-derived; cache keyed on the bases tuple).
  - Primary path: gpsimd.dma_gather (SWDGE custom op) pulls NT=384 tokens
    per call — the ~1us SWDGE cost is per CALL, so 11 calls replace the 33
    serial indirect DMAs that made the old gather stream ~46us. Indices are
    int16 relative to the chunk's 32768-row window; table rows stored bf16
    padded to 128 cols (256B, the dma_gather granularity).
  - Per 128-token block: PE transpose (64 cols) into a shared [128,128]
    PSUM tile (two blocks -> two PE row groups so their projection matmuls
    stream concurrently), copy to SBUF, 2 matmuls into a 2-bank PSUM tile
    [128,1024] f32, ONE cast to bf16 (alternating vector/scalar engines),
    store. bf16 output halves the dominant HBM write vs f32 (~1e-3 rel err,
    far inside the 2e-2 gate). Host scatters slabs back to token order.
  - Fallback (any window infeasible): per-128-token-block HW indirect DMA
    (int32 ids over the whole shard) — slower but unconditionally correct.
"""

import os as _os
from contextlib import ExitStack

import ml_dtypes
import numpy as np

import concourse.bass as bass
import concourse.mybir as mybir
import concourse.tile as tile
from concourse import bacc
from concourse.bass import IndirectOffsetOnAxis
from concourse.bass_utils import run_bass_kernel_spmd

VARIANT = _os.environ.get("BIGRAM_VARIANT", "dg")  # "dg" | "ind"
NT = int(_os.environ.get("BIGRAM_NT", "384"))  # tokens per dma_gather call

NUM_BUCKETS = 2000003
HASH_DIM = 64
EPAD = 128  # padded row length (bf16) for the 256B dma_gather granularity
MODEL_DIM = 1024
HASH_MULT = 92821
N_CORES = 8
P = 128
SHARD = 250001  # ceil(NUM_BUCKETS / N_CORES); 8*250001 = 2000008 >= NUM_BUCKETS
NFREE = 512  # matmul free dim (one PSUM bank of f32)
W = 32768  # gather window rows (int16 index range)

_prog_cache: dict = {}


def _build_dg_program(K: int, bases: tuple) -> "bacc.Bacc":
    """dma_gather path: chunks of NT tokens (NT/128 blocks) per SWDGE call."""
    nts = []
    r = K * P
    while r > 0:
        nts.append(min(NT, r))
        r -= nts[-1]
    NCH = len(nts)
    assert len(bases) == NCH
    nc = bacc.Bacc(
        "TRN2",
        target_bir_lowering=False,
        debug=False,
        num_devices=N_CORES,
        dynamic_dma_scratch_size=65536,
    )
    f32 = mybir.dt.float32
    bf16 = mybir.dt.bfloat16
    ncols = (K * P) // 16
    idx_d = nc.dram_tensor(
        "idx16", [P, ncols], mybir.dt.int16, kind="ExternalInput"
    ).ap()
    tab_d = nc.dram_tensor("table", [SHARD, EPAD], bf16, kind="ExternalInput").ap()
    projT_d = nc.dram_tensor(
        "projT", [HASH_DIM, MODEL_DIM], bf16, kind="ExternalInput"
    ).ap()
    ident_d = nc.dram_tensor("ident", [P, P], bf16, kind="ExternalInput").ap()
    out_d = nc.dram_tensor("out", [P * K, MODEL_DIM], bf16, kind="ExternalOutput").ap()

    # block -> (chunk, sub-block) map; chunk col offsets into idx16
    blk_ch, blk_sub, col_off = [], [], []
    off = 0
    for ch, nt in enumerate(nts):
        col_off.append(off)
        off += nt // 16
        for s in range(nt // P):
            blk_ch.append(ch)
            blk_sub.append(s)
    assert len(blk_ch) == K

    with tile.TileContext(nc) as tc, ExitStack() as ctx:
        const_p = ctx.enter_context(tc.tile_pool(name="const", bufs=1))
        idx_p = ctx.enter_context(tc.tile_pool(name="idx", bufs=1))
        emb_p = ctx.enter_context(tc.tile_pool(name="emb", bufs=6))
        embT_p = ctx.enter_context(tc.tile_pool(name="embT", bufs=4))
        out_p = ctx.enter_context(tc.tile_pool(name="out", bufs=6))
        ps_t = ctx.enter_context(tc.tile_pool(name="ps_t", bufs=2, space="PSUM"))
        # PSUM is bank-granular: ps_t 2x1 bank + ps_mm 3x2 banks = all 8.
        ps_mm = ctx.enter_context(tc.tile_pool(name="ps_mm", bufs=3, space="PSUM"))

        # If K is odd, gather the odd block's chunk first and process the
        # odd block (serial matmul chain, no row-group partner) in the ramp.
        ch_order = list(range(NCH))
        pairs = [[2 * q, 2 * q + 1] for q in range(K // 2)]
        if K % 2:
            ch_order = [NCH - 1] + ch_order[:-1]
            pairs = [[K - 1]] + pairs

        # idx first: the gather stream depends only on it. Load the first
        # gathered chunk's columns separately so gather 0 is ungated fast.
        idx_t = idx_p.tile([P, ncols], mybir.dt.int16)
        f_lo = col_off[ch_order[0]]
        f_hi = f_lo + nts[ch_order[0]] // 16
        nc.sync.dma_start(out=idx_t[:, f_lo:f_hi], in_=idx_d[:, f_lo:f_hi])
        if f_lo > 0:
            nc.sync.dma_start(out=idx_t[:, :f_lo], in_=idx_d[:, :f_lo])
        if f_hi < ncols:
            nc.sync.dma_start(out=idx_t[:, f_hi:], in_=idx_d[:, f_hi:])
        ident = const_p.tile([P, P], bf16)
        nc.sync.dma_start(out=ident[:], in_=ident_d[:])
        # projT duplicated on partitions 0-63 / 64-127 so a pair's matmuls
        # run concurrently in separate PE row groups (auto tile_position).
        projT_s = const_p.tile([P, MODEL_DIM], bf16)
        nc.sync.dma_start(out=projT_s[:HASH_DIM, :], in_=projT_d[:])
        nc.sync.dma_start(out=projT_s[HASH_DIM:, :], in_=projT_d[:])
        # PE warm-up during the DMA-wait ramp: releases the HAM clock gate
        # before the first real transpose.
        dumw = const_p.tile([P, NFREE], bf16)
        nc.vector.memset(dumw[:], 0.0)
        wps = ps_mm.tile([P, MODEL_DIM], f32, name="mm")
        for _ in range(8):
            nc.tensor.matmul(
                wps[:, :NFREE], lhsT=dumw[:, :P], rhs=dumw[:], start=True, stop=True
            )

        emb_tiles = {}
        for ch in ch_order:
            nt = nts[ch]
            nb = nt // P
            embp = emb_p.tile([P, nb * EPAD], bf16, name="embp")
            emb3 = embp[:].rearrange("p (c e) -> p c e", e=EPAD)
            # single_packet=False: descriptors split across all 16 SDMA
            # engines — single_packet pins the whole chunk's 98KB on ONE
            # engine (~27 GB/s), which made each call ~3.6us on HW.
            nc.gpsimd.dma_gather(
                emb3,
                tab_d[bases[ch] : bases[ch] + W, :],
                idx_t[:, col_off[ch] : col_off[ch] + nt // 16],
                nt,
                nt,
                EPAD,
                single_packet=False,
            )
            emb_tiles[ch] = embp

        cast_i = 0
        for pair in pairs:
            nb = len(pair)
            eT_ps = ps_t.tile([nb * HASH_DIM, P], bf16, name="eT_ps")
            for jj, b in enumerate(pair):
                nc.tensor.transpose(
                    eT_ps[jj * HASH_DIM : (jj + 1) * HASH_DIM, :],
                    emb_tiles[blk_ch[b]][
                        :, blk_sub[b] * EPAD : blk_sub[b] * EPAD + HASH_DIM
                    ],
                    ident[:],
                )
            eT = embT_p.tile([nb * HASH_DIM, P], bf16, name="eT")
            # Alternate the eT copies too so vector/scalar stay balanced.
            if cast_i % 2 == 0:
                nc.vector.tensor_copy(eT[:], eT_ps[:])
            else:
                nc.scalar.copy(eT[:], eT_ps[:])
            mms = [ps_mm.tile([P, MODEL_DIM], f32, name="mm") for _ in range(nb)]
            # Interleave matmuls across the two blocks so consecutive PE
            # instructions target alternating row groups and overlap.
            for h in range(MODEL_DIM // NFREE):
                for jj in range(nb):
                    nc.tensor.matmul(
                        mms[jj][:, h * NFREE : (h + 1) * NFREE],
                        lhsT=eT[jj * HASH_DIM : (jj + 1) * HASH_DIM, :],
                        rhs=projT_s[
                            jj * HASH_DIM : (jj + 1) * HASH_DIM,
                            h * NFREE : (h + 1) * NFREE,
                        ],
                        start=True,
                        stop=True,
                    )
            for jj, b in enumerate(pair):
                o_t = out_p.tile([P, MODEL_DIM], bf16, name="o_t")
                if (cast_i + jj) % 2 == 0:
                    nc.vector.tensor_copy(o_t[:], mms[jj][:])
                else:
                    nc.scalar.copy(o_t[:], mms[jj][:])
                nc.sync.dma_start(out=out_d[b * P : (b + 1) * P, :], in_=o_t[:])
            cast_i += 1
    nc.compile()
    return nc


def _build_ind_program(K: int) -> "bacc.Bacc":
    """Fallback: per-128-token-block indirect DMA gather (int32 ids).

    HW semantics allow only ONE offset per partition per call, so this path
    costs ~1.4us of gpsimd per 128 tokens — correct for any input, slow."""
    nc = bacc.Bacc(
        "TRN2",
        target_bir_lowering=False,
        debug=False,
        num_devices=N_CORES,
        dynamic_dma_scratch_size=65536,
    )
    f32 = mybir.dt.float32
    bf16 = mybir.dt.bfloat16
    idx_d = nc.dram_tensor("idx", [P, K], mybir.dt.int32, kind="ExternalInput").ap()
    tab_d = nc.dram_tensor("table", [SHARD, HASH_DIM], bf16, kind="ExternalInput").ap()
    projT_d = nc.dram_tensor(
        "projT", [HASH_DIM, MODEL_DIM], bf16, kind="ExternalInput"
    ).ap()
    ident_d = nc.dram_tensor("ident", [P, P], bf16, kind="ExternalInput").ap()
    out_d = nc.dram_tensor("out", [P * K, MODEL_DIM], bf16, kind="ExternalOutput").ap()

    with tile.TileContext(nc) as tc, ExitStack() as ctx:
        const_p = ctx.enter_context(tc.tile_pool(name="const", bufs=1))
        idx_p = ctx.enter_context(tc.tile_pool(name="idx", bufs=1))
        emb_p = ctx.enter_context(tc.tile_pool(name="emb", bufs=6))
        embT_p = ctx.enter_context(tc.tile_pool(name="embT", bufs=4))
        out_p = ctx.enter_context(tc.tile_pool(name="out", bufs=6))
        ps_t = ctx.enter_context(tc.tile_pool(name="ps_t", bufs=2, space="PSUM"))
        ps_mm = ctx.enter_context(tc.tile_pool(name="ps_mm", bufs=3, space="PSUM"))

        idx_t = idx_p.tile([P, K], mybir.dt.int32)
        nc.sync.dma_start(out=idx_t[:], in_=idx_d[:])
        ident = const_p.tile([P, P], bf16)
        nc.sync.dma_start(out=ident[:], in_=ident_d[:])
        projT_s = const_p.tile([P, MODEL_DIM], bf16)
        nc.sync.dma_start(out=projT_s[:HASH_DIM, :], in_=projT_d[:])
        nc.sync.dma_start(out=projT_s[HASH_DIM:, :], in_=projT_d[:])

        pbs = list(range(0, K, 2))
        if K % 2:
            pbs = pbs[-1:] + pbs[:-1]
        cast_i = 0
        for pb in pbs:
            nblocks = min(2, K - pb)
            embp = emb_p.tile([P, nblocks * HASH_DIM], bf16)
            for j in range(nblocks):
                nc.gpsimd.indirect_dma_start(
                    out=embp[:, j * HASH_DIM : (j + 1) * HASH_DIM],
                    out_offset=None,
                    in_=tab_d[:],
                    in_offset=IndirectOffsetOnAxis(
                        ap=idx_t[:, pb + j : pb + j + 1], axis=0
                    ),
                )
            eT_ps = ps_t.tile([nblocks * HASH_DIM, P], bf16)
            nc.tensor.transpose(eT_ps[:], embp[:], ident[:])
            eT = embT_p.tile([nblocks * HASH_DIM, P], bf16)
            if cast_i % 2 == 0:
                nc.vector.tensor_copy(eT[:], eT_ps[:])
            else:
                nc.scalar.copy(eT[:], eT_ps[:])
            mms = [ps_mm.tile([P, MODEL_DIM], f32, name="mm") for _ in range(nblocks)]
            for h in range(MODEL_DIM // NFREE):
                for jj in range(nblocks):
                    nc.tensor.matmul(
                        mms[jj][:, h * NFREE : (h + 1) * NFREE],
                        lhsT=eT[jj * HASH_DIM : (jj + 1) * HASH_DIM, :],
                        rhs=projT_s[
                            jj * HASH_DIM : (jj + 1) * HASH_DIM,
                            h * NFREE : (h + 1) * NFREE,
                        ],
                        start=True,
                        stop=True,
                    )
            for jj in range(nblocks):
                o_t = out_p.tile([P, MODEL_DIM], bf16, name="o_t")
                if (cast_i + jj) % 2 == 0:
                    nc.vector.tensor_copy(o_t[:], mms[jj][:])
                else:
                    nc.scalar.copy(o_t[:], mms[jj][:])
                nc.sync.dma_start(
                    out=out_d[(pb + jj) * P : (pb + jj + 1) * P, :], in_=o_t[:]
                )
            cast_i += 1
    nc.compile()
    return nc


def prepare(input_ids, table, proj_w):
    """Route tokens, pick program variant, build per-core in_maps."""
    B, S = input_ids.shape
    T = B * S
    ids = np.asarray(input_ids, dtype=np.int64)
    prev = np.empty_like(ids)
    prev[:, 0] = 0
    prev[:, 1:] = ids[:, :-1]
    h = ((prev * HASH_MULT + ids) % NUM_BUCKETS).reshape(-1)
    owner = h // SHARD
    local = (h - owner * SHARD).astype(np.int64)
    order = np.lexsort((local, owner))
    counts = np.bincount(owner, minlength=N_CORES).astype(np.int64)
    offsets = np.zeros(N_CORES + 1, dtype=np.int64)
    np.cumsum(counts, out=offsets[1:])
    sorted_local = local[order]

    cap = max(P, int(-(-counts.max() // P)) * P)
    K = cap // P
    nts = []
    r = cap
    while r > 0:
        nts.append(min(NT, r))
        r -= nts[-1]
    NCH = len(nts)
    cuts = np.zeros(NCH + 1, dtype=np.int64)
    np.cumsum(nts, out=cuts[1:])

    # Spread each core's real ids evenly over the cap slots; pads forward-
    # fill so the padded sequence stays sorted and chunk windows stay tight.
    pos_list, padded_list = [], []
    for c in range(N_CORES):
        loc = sorted_local[offsets[c] : offsets[c + 1]]
        n = len(loc)
        if n == 0:
            pos = np.zeros(0, dtype=np.int64)
            row = np.zeros(cap, dtype=np.int64)
        else:
            pos = (np.arange(n, dtype=np.int64) * cap) // n
            row = np.zeros(cap, dtype=np.int64)
            row[pos] = loc
            mark = np.full(cap, -1, dtype=np.int64)
            mark[pos] = np.arange(cap, dtype=np.int64)[pos]
            np.maximum.accumulate(mark, out=mark)
            row = row[np.maximum(mark, 0)]
        pos_list.append(pos)
        padded_list.append(row)
    padded_all = np.stack(padded_list)  # [N_CORES, cap]

    # Exact cross-core window bases; feasibility per chunk.
    bases = []
    ok = VARIANT == "dg"
    for ch in range(NCH):
        lo = int(padded_all[:, cuts[ch]].min())
        hi = int(padded_all[:, cuts[ch + 1] - 1].max())
        b = min(max(lo, 0), SHARD - W)
        bases.append(b)
        if hi - b > W - 1:
            ok = False
    bases = tuple(bases)

    table = np.asarray(table, dtype=np.float32)
    projT = np.ascontiguousarray(
        np.asarray(proj_w, dtype=np.float32).T.astype(ml_dtypes.bfloat16)
    )
    in_maps = []
    for c in range(N_CORES):
        lo, hi = c * SHARD, min((c + 1) * SHARD, NUM_BUCKETS)
        ncols_tab = EPAD if ok else HASH_DIM
        shard = np.zeros((SHARD, ncols_tab), dtype=ml_dtypes.bfloat16)
        shard[: hi - lo, :HASH_DIM] = table[lo:hi].astype(ml_dtypes.bfloat16)
        m = {"table": shard, "projT": projT}
        if ok:
            # idx16[p, col_off+s] = chunk token s*16+p, relative to the
            # chunk base; wrapped in 16 partitions, replicated to all 8
            # 16-partition Q7 core groups.
            rel = padded_all[c].copy()
            for ch in range(NCH):
                rel[cuts[ch] : cuts[ch + 1]] -= bases[ch]
            rel = np.maximum(rel, 0)
            cols = [
                rel[cuts[ch] : cuts[ch + 1]].reshape(-1, 16).T for ch in range(NCH)
            ]
            row16 = np.concatenate(cols, axis=1).astype(np.int16)
            m["idx16"] = np.ascontiguousarray(np.tile(row16, (P // 16, 1)))
        else:
            padded = np.zeros(cap, dtype=np.int64)
            padded[: counts[c]] = sorted_local[offsets[c] : offsets[c + 1]]
            m["idx"] = np.ascontiguousarray(padded.astype(np.int32).reshape(K, P).T)
        m["ident"] = np.eye(P, dtype=ml_dtypes.bfloat16)
        in_maps.append(m)

    key = ("dg", K, NT, bases) if ok else ("ind", K)
    nc = _prog_cache.get(key)
    if nc is None:
        nc = _build_dg_program(K, bases) if ok else _build_ind_program(K)
        _prog_cache[key] = nc
    # row_map[c]: device out row holding sorted token k of core c
    if ok:
        row_map = pos_list
    else:
        row_map = [np.arange(counts[c], dtype=np.int64) for c in range(N_CORES)]
    meta = (T, order, offsets, row_map, K)
    return nc, in_maps, meta


def kernel(input_ids: np.ndarray, table: np.ndarray, proj_w: np.ndarray) -> np.ndarray:
    B, S = input_ids.shape
    nc, in_maps, meta = prepare(input_ids, table, proj_w)
    T, order, offsets, row_map, K = meta
    res = run_bass_kernel_spmd(nc, in_maps, list(range(N_CORES)))
    flat = np.empty((T, MODEL_DIM), dtype=np.float32)
    for c in range(N_CORES):
        flat[order[offsets[c] : offsets[c + 1]]] = res.results[c]["out"][
            row_map[c]
        ].astype(np.float32)
    return flat.reshape(B, S, MODEL_DIM)


# revision 16
# speedup vs baseline: 1.0078x; 1.0038x over previous
"""BigramHash embedding lookup kernel for 8 Trainium2 NeuronCores.

Strategy (row-sharded table, host-side all-to-all since we receive full inputs):
  - Host computes bucket ids h = (prev_id * MULT + id) % NUM_BUCKETS, routes
    each token to the core owning its table shard (SHARD = 250001 rows), and
    sorts by local row id (HBM locality + windowed int16 gather indices).
  - Real tokens are spread evenly over the padded cap slots (pads forward-
    fill the previous id) so position-aligned gather chunks cover the same
    id quantiles on every core — the SPMD program bakes ONE window base per
    chunk, taken as the cross-core min of the chunk's first id (exact,
    data-derived; cache keyed on the bases tuple).
  - Primary path: gpsimd.dma_gather (SWDGE custom op) pulls NT=384 tokens
    per call — the ~1us SWDGE cost is per CALL, so 11 calls replace the 33
    serial indirect DMAs that made the old gather stream ~46us. Indices are
    int16 relative to the chunk's 32768-row window; table rows stored bf16
    padded to 128 cols (256B, the dma_gather granularity).
  - Per 128-token block: PE transpose (64 cols) into a shared [128,128]
    PSUM tile (two blocks -> two PE row groups so their projection matmuls
    stream concurrently), copy to SBUF, 2 matmuls into a 2-bank PSUM tile
    [128,1024] f32, ONE cast to bf16 (alternating vector/scalar engines),
    store. bf16 output halves the dominant HBM write vs f32 (~1e-3 rel err,
    far inside the 2e-2 gate). Host scatters slabs back to token order.
  - Fallback (any window infeasible): per-128-token-block HW indirect DMA
    (int32 ids over the whole shard) — slower but unconditionally correct.
"""

import os as _os
from contextlib import ExitStack

import ml_dtypes
import numpy as np

import concourse.bass as bass
import concourse.mybir as mybir
import concourse.tile as tile
from concourse import bacc
from concourse.bass import IndirectOffsetOnAxis
from concourse.bass_utils import run_bass_kernel_spmd

VARIANT = _os.environ.get("BIGRAM_VARIANT", "dg")  # "dg" | "ind"
NT = int(_os.environ.get("BIGRAM_NT", "512"))  # max tokens per dma_gather call

NUM_BUCKETS = 2000003
HASH_DIM = 64
EPAD = 128  # padded row length (bf16) for the 256B dma_gather granularity
MODEL_DIM = 1024
HASH_MULT = 92821
N_CORES = 8
P = 128
SHARD = 250001  # ceil(NUM_BUCKETS / N_CORES); 8*250001 = 2000008 >= NUM_BUCKETS
NFREE = 512  # matmul free dim (one PSUM bank of f32)
W = 32768  # gather window rows (int16 index range)

_prog_cache: dict = {}


def _build_dg_program(K: int, bases: tuple, nts: tuple) -> "bacc.Bacc":
    """dma_gather path: greedy variable-size chunks, one SWDGE call each."""
    NCH = len(nts)
    assert len(bases) == NCH and sum(nts) == K * P
    nc = bacc.Bacc(
        "TRN2",
        target_bir_lowering=False,
        debug=False,
        num_devices=N_CORES,
        dynamic_dma_scratch_size=65536,
    )
    f32 = mybir.dt.float32
    bf16 = mybir.dt.bfloat16
    ncols = (K * P) // 16
    idx_d = nc.dram_tensor(
        "idx16", [P, ncols], mybir.dt.int16, kind="ExternalInput"
    ).ap()
    tab_d = nc.dram_tensor("table", [SHARD, EPAD], bf16, kind="ExternalInput").ap()
    projT_d = nc.dram_tensor(
        "projT", [HASH_DIM, MODEL_DIM], bf16, kind="ExternalInput"
    ).ap()
    ident_d = nc.dram_tensor("ident", [P, P], bf16, kind="ExternalInput").ap()
    out_d = nc.dram_tensor("out", [P * K, MODEL_DIM], bf16, kind="ExternalOutput").ap()

    # block -> (chunk, sub-block) map; chunk col offsets into idx16
    blk_ch, blk_sub, col_off = [], [], []
    off = 0
    for ch, nt in enumerate(nts):
        col_off.append(off)
        off += nt // 16
        for s in range(nt // P):
            blk_ch.append(ch)
            blk_sub.append(s)
    assert len(blk_ch) == K

    with tile.TileContext(nc) as tc, ExitStack() as ctx:
        const_p = ctx.enter_context(tc.tile_pool(name="const", bufs=1))
        idx_p = ctx.enter_context(tc.tile_pool(name="idx", bufs=1))
        emb_p = ctx.enter_context(tc.tile_pool(name="emb", bufs=6))
        embT_p = ctx.enter_context(tc.tile_pool(name="embT", bufs=6))
        out_p = ctx.enter_context(tc.tile_pool(name="out", bufs=8))
        ps_t = ctx.enter_context(tc.tile_pool(name="ps_t", bufs=2, space="PSUM"))
        # PSUM is bank-granular: ps_t 2x1 bank + ps_mm 6x1 bank = all 8.
        # Six 1-bank matmul tiles decouple a pair's matmuls from the
        # previous pair's cast completions (2-bank x3 serialized the tail).
        ps_mm = ctx.enter_context(tc.tile_pool(name="ps_mm", bufs=6, space="PSUM"))

        # Block 0 sits alone in chunk 0 (the odd block when K is odd runs
        # in the PE ramp); all later blocks pair for row-group overlap.
        pairs = [[0]] + [[1 + 2 * q, 2 + 2 * q] for q in range((K - 1) // 2)]
        if K % 2 == 0 and K > 1:
            pairs.append([K - 1])

        # idx first: the gather stream depends only on it. Load the first
        # chunk's columns separately so gather 0 is ungated fast.
        idx_t = idx_p.tile([P, ncols], mybir.dt.int16)
        f_hi = nts[0] // 16
        nc.sync.dma_start(out=idx_t[:, :f_hi], in_=idx_d[:, :f_hi])
        if f_hi < ncols:
            nc.sync.dma_start(out=idx_t[:, f_hi:], in_=idx_d[:, f_hi:])
        ident = const_p.tile([P, P], bf16)
        nc.sync.dma_start(out=ident[:], in_=ident_d[:])
        # projT duplicated on partitions 0-63 / 64-127 so a pair's matmuls
        # run concurrently in separate PE row groups (auto tile_position).
        projT_s = const_p.tile([P, MODEL_DIM], bf16)
        nc.sync.dma_start(out=projT_s[:HASH_DIM, :], in_=projT_d[:])
        nc.sync.dma_start(out=projT_s[HASH_DIM:, :], in_=projT_d[:])
        # PE warm-up during the DMA-wait ramp: releases the HAM clock gate
        # before the first real transpose.
        dumw = const_p.tile([P, NFREE], bf16)
        nc.vector.memset(dumw[:], 0.0)
        wps = ps_mm.tile([P, NFREE], f32, name="mm")
        for _ in range(8):
            nc.tensor.matmul(
                wps[:], lhsT=dumw[:, :P], rhs=dumw[:], start=True, stop=True
            )

        emb_tiles = {}
        for ch in range(NCH):
            nt = nts[ch]
            nb = nt // P
            embp = emb_p.tile([P, nb * EPAD], bf16, name="embp")
            emb3 = embp[:].rearrange("p (c e) -> p c e", e=EPAD)
            nc.gpsimd.dma_gather(
                emb3,
                tab_d[bases[ch] : bases[ch] + W, :],
                idx_t[:, col_off[ch] : col_off[ch] + nt // 16],
                nt,
                nt,
                EPAD,
                single_packet=False,
            )
            emb_tiles[ch] = embp

        cast_i = 0
        for pair in pairs:
            nb = len(pair)
            eT_ps = ps_t.tile([nb * HASH_DIM, P], bf16, name="eT_ps")
            for jj, b in enumerate(pair):
                nc.tensor.transpose(
                    eT_ps[jj * HASH_DIM : (jj + 1) * HASH_DIM, :],
                    emb_tiles[blk_ch[b]][
                        :, blk_sub[b] * EPAD : blk_sub[b] * EPAD + HASH_DIM
                    ],
                    ident[:],
                )
            eT = embT_p.tile([nb * HASH_DIM, P], bf16, name="eT")
            # Alternate the eT copies too so vector/scalar stay balanced.
            if cast_i % 2 == 0:
                nc.vector.tensor_copy(eT[:], eT_ps[:])
            else:
                nc.scalar.copy(eT[:], eT_ps[:])
            o_t = [out_p.tile([P, MODEL_DIM], bf16, name="o_t") for _ in range(nb)]
            # Interleave matmuls across the two blocks so consecutive PE
            # instructions target alternating row groups and overlap; cast
            # each 1-bank PSUM tile right after its matmul so banks recycle.
            for h in range(MODEL_DIM // NFREE):
                mms = [ps_mm.tile([P, NFREE], f32, name="mm") for _ in range(nb)]
                for jj in range(nb):
                    nc.tensor.matmul(
                        mms[jj][:],
                        lhsT=eT[jj * HASH_DIM : (jj + 1) * HASH_DIM, :],
                        rhs=projT_s[
                            jj * HASH_DIM : (jj + 1) * HASH_DIM,
                            h * NFREE : (h + 1) * NFREE,
                        ],
                        start=True,
                        stop=True,
                    )
                for jj in range(nb):
                    dst = o_t[jj][:, h * NFREE : (h + 1) * NFREE]
                    if (cast_i + jj) % 2 == 0:
                        nc.vector.tensor_copy(dst, mms[jj][:])
                    else:
                        nc.scalar.copy(dst, mms[jj][:])
            for jj, b in enumerate(pair):
                nc.sync.dma_start(out=out_d[b * P : (b + 1) * P, :], in_=o_t[jj][:])
            cast_i += 1
    nc.compile()
    return nc


def _build_ind_program(K: int) -> "bacc.Bacc":
    """Fallback: per-128-token-block indirect DMA gather (int32 ids).

    HW semantics allow only ONE offset per partition per call, so this path
    costs ~1.4us of gpsimd per 128 tokens — correct for any input, slow."""
    nc = bacc.Bacc(
        "TRN2",
        target_bir_lowering=False,
        debug=False,
        num_devices=N_CORES,
        dynamic_dma_scratch_size=65536,
    )
    f32 = mybir.dt.float32
    bf16 = mybir.dt.bfloat16
    idx_d = nc.dram_tensor("idx", [P, K], mybir.dt.int32, kind="ExternalInput").ap()
    tab_d = nc.dram_tensor("table", [SHARD, HASH_DIM], bf16, kind="ExternalInput").ap()
    projT_d = nc.dram_tensor(
        "projT", [HASH_DIM, MODEL_DIM], bf16, kind="ExternalInput"
    ).ap()
    ident_d = nc.dram_tensor("ident", [P, P], bf16, kind="ExternalInput").ap()
    out_d = nc.dram_tensor("out", [P * K, MODEL_DIM], bf16, kind="ExternalOutput").ap()

    with tile.TileContext(nc) as tc, ExitStack() as ctx:
        const_p = ctx.enter_context(tc.tile_pool(name="const", bufs=1))
        idx_p = ctx.enter_context(tc.tile_pool(name="idx", bufs=1))
        emb_p = ctx.enter_context(tc.tile_pool(name="emb", bufs=6))
        embT_p = ctx.enter_context(tc.tile_pool(name="embT", bufs=4))
        out_p = ctx.enter_context(tc.tile_pool(name="out", bufs=6))
        ps_t = ctx.enter_context(tc.tile_pool(name="ps_t", bufs=2, space="PSUM"))
        ps_mm = ctx.enter_context(tc.tile_pool(name="ps_mm", bufs=3, space="PSUM"))

        idx_t = idx_p.tile([P, K], mybir.dt.int32)
        nc.sync.dma_start(out=idx_t[:], in_=idx_d[:])
        ident = const_p.tile([P, P], bf16)
        nc.sync.dma_start(out=ident[:], in_=ident_d[:])
        projT_s = const_p.tile([P, MODEL_DIM], bf16)
        nc.sync.dma_start(out=projT_s[:HASH_DIM, :], in_=projT_d[:])
        nc.sync.dma_start(out=projT_s[HASH_DIM:, :], in_=projT_d[:])

        pbs = list(range(0, K, 2))
        if K % 2:
            pbs = pbs[-1:] + pbs[:-1]
        cast_i = 0
        for pb in pbs:
            nblocks = min(2, K - pb)
            embp = emb_p.tile([P, nblocks * HASH_DIM], bf16)
            for j in range(nblocks):
                nc.gpsimd.indirect_dma_start(
                    out=embp[:, j * HASH_DIM : (j + 1) * HASH_DIM],
                    out_offset=None,
                    in_=tab_d[:],
                    in_offset=IndirectOffsetOnAxis(
                        ap=idx_t[:, pb + j : pb + j + 1], axis=0
                    ),
                )
            eT_ps = ps_t.tile([nblocks * HASH_DIM, P], bf16)
            nc.tensor.transpose(eT_ps[:], embp[:], ident[:])
            eT = embT_p.tile([nblocks * HASH_DIM, P], bf16)
            if cast_i % 2 == 0:
                nc.vector.tensor_copy(eT[:], eT_ps[:])
            else:
                nc.scalar.copy(eT[:], eT_ps[:])
            mms = [ps_mm.tile([P, MODEL_DIM], f32, name="mm") for _ in range(nblocks)]
            for h in range(MODEL_DIM // NFREE):
                for jj in range(nblocks):
                    nc.tensor.matmul(
                        mms[jj][:, h * NFREE : (h + 1) * NFREE],
                        lhsT=eT[jj * HASH_DIM : (jj + 1) * HASH_DIM, :],
                        rhs=projT_s[
                            jj * HASH_DIM : (jj + 1) * HASH_DIM,
                            h * NFREE : (h + 1) * NFREE,
                        ],
                        start=True,
                        stop=True,
                    )
            for jj in range(nblocks):
                o_t = out_p.tile([P, MODEL_DIM], bf16, name="o_t")
                if (cast_i + jj) % 2 == 0:
                    nc.vector.tensor_copy(o_t[:], mms[jj][:])
                else:
                    nc.scalar.copy(o_t[:], mms[jj][:])
                nc.sync.dma_start(
                    out=out_d[(pb + jj) * P : (pb + jj + 1) * P, :], in_=o_t[:]
                )
            cast_i += 1
    nc.compile()
    return nc


def prepare(input_ids, table, proj_w):
    """Route tokens, pick program variant, build per-core in_maps."""
    B, S = input_ids.shape
    T = B * S
    ids = np.asarray(input_ids, dtype=np.int64)
    prev = np.empty_like(ids)
    prev[:, 0] = 0
    prev[:, 1:] = ids[:, :-1]
    h = ((prev * HASH_MULT + ids) % NUM_BUCKETS).reshape(-1)
    owner = h // SHARD
    local = (h - owner * SHARD).astype(np.int64)
    order = np.lexsort((local, owner))
    counts = np.bincount(owner, minlength=N_CORES).astype(np.int64)
    offsets = np.zeros(N_CORES + 1, dtype=np.int64)
    np.cumsum(counts, out=offsets[1:])
    sorted_local = local[order]

    cap = max(P, int(-(-counts.max() // P)) * P)
    K = cap // P

    # Spread each core's real ids evenly over the cap slots; pads forward-
    # fill so the padded sequence stays sorted and chunk windows stay tight.
    pos_list, padded_list = [], []
    for c in range(N_CORES):
        loc = sorted_local[offsets[c] : offsets[c + 1]]
        n = len(loc)
        if n == 0:
            pos = np.zeros(0, dtype=np.int64)
            row = np.zeros(cap, dtype=np.int64)
        else:
            pos = (np.arange(n, dtype=np.int64) * cap) // n
            row = np.zeros(cap, dtype=np.int64)
            row[pos] = loc
            mark = np.full(cap, -1, dtype=np.int64)
            mark[pos] = np.arange(cap, dtype=np.int64)[pos]
            np.maximum.accumulate(mark, out=mark)
            row = row[np.maximum(mark, 0)]
        pos_list.append(pos)
        padded_list.append(row)
    padded_all = np.stack(padded_list)  # [N_CORES, cap]

    # Greedy variable chunks (multiples of 128 tokens, <= NT) with exact
    # cross-core window bases. Chunk 0 is a single block so the odd/solo
    # block runs in the PE ramp and every later block pairs up.
    lo_all = padded_all.min(axis=0)
    hi_all = padded_all.max(axis=0)
    nts, bases = [], []
    ok = VARIANT == "dg"
    start = 0
    while start < cap and ok:
        b = min(max(int(lo_all[start]), 0), SHARD - W)
        limit = P if start == 0 else min(NT, cap - start)
        size = 0
        for step in range(P, limit + P, P):
            if start + step > cap:
                break
            if int(hi_all[start + step - 1]) - b <= W - 1:
                size = step
            else:
                break
        if size == 0:
            ok = False
            break
        nts.append(size)
        bases.append(b)
        start += size
    if not ok:
        nts, bases = [cap], [0]
    NCH = len(nts)
    cuts = np.zeros(NCH + 1, dtype=np.int64)
    np.cumsum(nts, out=cuts[1:])
    bases = tuple(bases)
    nts = tuple(nts)

    table = np.asarray(table, dtype=np.float32)
    projT = np.ascontiguousarray(
        np.asarray(proj_w, dtype=np.float32).T.astype(ml_dtypes.bfloat16)
    )
    in_maps = []
    for c in range(N_CORES):
        lo, hi = c * SHARD, min((c + 1) * SHARD, NUM_BUCKETS)
        ncols_tab = EPAD if ok else HASH_DIM
        shard = np.zeros((SHARD, ncols_tab), dtype=ml_dtypes.bfloat16)
        shard[: hi - lo, :HASH_DIM] = table[lo:hi].astype(ml_dtypes.bfloat16)
        m = {"table": shard, "projT": projT}
        if ok:
            # idx16[p, col_off+s] = chunk token s*16+p, relative to the
            # chunk base; wrapped in 16 partitions, replicated to all 8
            # 16-partition Q7 core groups.
            rel = padded_all[c].copy()
            for ch in range(NCH):
                rel[cuts[ch] : cuts[ch + 1]] -= bases[ch]
            rel = np.maximum(rel, 0)
            cols = [
                rel[cuts[ch] : cuts[ch + 1]].reshape(-1, 16).T for ch in range(NCH)
            ]
            row16 = np.concatenate(cols, axis=1).astype(np.int16)
            m["idx16"] = np.ascontiguousarray(np.tile(row16, (P // 16, 1)))
        else:
            padded = np.zeros(cap, dtype=np.int64)
            padded[: counts[c]] = sorted_local[offsets[c] : offsets[c + 1]]
            m["idx"] = np.ascontiguousarray(padded.astype(np.int32).reshape(K, P).T)
        m["ident"] = np.eye(P, dtype=ml_dtypes.bfloat16)
        in_maps.append(m)

    key = ("dg", K, nts, bases) if ok else ("ind", K)
    nc = _prog_cache.get(key)
    if nc is None:
        nc = _build_dg_program(K, bases, nts) if ok else _build_ind_program(K)
        _prog_cache[key] = nc
    # row_map[c]: device out row holding sorted token k of core c
    if ok:
        row_map = pos_list
    else:
        row_map = [np.arange(counts[c], dtype=np.int64) for c in range(N_CORES)]
    meta = (T, order, offsets, row_map, K)
    return nc, in_maps, meta


def kernel(input_ids: np.ndarray, table: np.ndarray, proj_w: np.ndarray) -> np.ndarray:
    B, S = input_ids.shape
    nc, in_maps, meta = prepare(input_ids, table, proj_w)
    T, order, offsets, row_map, K = meta
    res = run_bass_kernel_spmd(nc, in_maps, list(range(N_CORES)))
    flat = np.empty((T, MODEL_DIM), dtype=np.float32)
    for c in range(N_CORES):
        flat[order[offsets[c] : offsets[c + 1]]] = res.results[c]["out"][
            row_map[c]
        ].astype(np.float32)
    return flat.reshape(B, S, MODEL_DIM)


# revision 17
# speedup vs baseline: 1.0703x; 1.0621x over previous
"""BigramHash embedding lookup kernel for 8 Trainium2 NeuronCores.

Strategy (row-sharded table, host-side all-to-all since we receive full inputs):
  - Host computes bucket ids h = (prev_id * MULT + id) % NUM_BUCKETS, routes
    each token to the core owning its table shard (SHARD = 250001 rows), and
    sorts by local row id (HBM locality + windowed int16 gather indices).
  - Real tokens are spread evenly over the padded cap slots (pads forward-
    fill the previous id) so position-aligned gather chunks cover the same
    id quantiles on every core — the SPMD program bakes ONE window base per
    chunk, taken as the cross-core min of the chunk's first id (exact,
    data-derived; cache keyed on the bases tuple).
  - Primary path: gpsimd.dma_gather (SWDGE custom op) pulls NT=384 tokens
    per call — the ~1us SWDGE cost is per CALL, so 11 calls replace the 33
    serial indirect DMAs that made the old gather stream ~46us. Indices are
    int16 relative to the chunk's 32768-row window; table rows stored bf16
    padded to 128 cols (256B, the dma_gather granularity).
  - Per 128-token block: PE transpose (64 cols) into a shared [128,128]
    PSUM tile (two blocks -> two PE row groups so their projection matmuls
    stream concurrently), copy to SBUF, 2 matmuls into a 2-bank PSUM tile
    [128,1024] f32, ONE cast to bf16 (alternating vector/scalar engines),
    store. bf16 output halves the dominant HBM write vs f32 (~1e-3 rel err,
    far inside the 2e-2 gate). Host scatters slabs back to token order.
  - Fallback (any window infeasible): per-128-token-block HW indirect DMA
    (int32 ids over the whole shard) — slower but unconditionally correct.
"""

import os as _os
from contextlib import ExitStack

import ml_dtypes
import numpy as np

import concourse.bass as bass
import concourse.mybir as mybir
import concourse.tile as tile
from concourse import bacc
from concourse.bass import IndirectOffsetOnAxis
from concourse.bass_utils import run_bass_kernel_spmd

VARIANT = _os.environ.get("BIGRAM_VARIANT", "dg")  # "dg" | "ind"
NT = int(_os.environ.get("BIGRAM_NT", "512"))  # max tokens per dma_gather call

NUM_BUCKETS = 2000003
HASH_DIM = 64
EPAD = 128  # padded row length (bf16) for the 256B dma_gather granularity
MODEL_DIM = 1024
HASH_MULT = 92821
N_CORES = 8
P = 128
SHARD = 250001  # ceil(NUM_BUCKETS / N_CORES); 8*250001 = 2000008 >= NUM_BUCKETS
NFREE = 512  # matmul free dim (one PSUM bank of f32)
W = 32768  # gather window rows (int16 index range)

_prog_cache: dict = {}


def _build_dg_program(K: int, bases: tuple, nts: tuple) -> "bacc.Bacc":
    """dma_gather path: greedy variable-size chunks, one SWDGE call each."""
    NCH = len(nts)
    assert len(bases) == NCH and sum(nts) == K * P
    nc = bacc.Bacc(
        "TRN2",
        target_bir_lowering=False,
        debug=False,
        num_devices=N_CORES,
        dynamic_dma_scratch_size=65536,
    )
    f32 = mybir.dt.float32
    bf16 = mybir.dt.bfloat16
    ncols = (K * P) // 16
    idx_d = nc.dram_tensor(
        "idx16", [P, ncols], mybir.dt.int16, kind="ExternalInput"
    ).ap()
    tab_d = nc.dram_tensor("table", [SHARD, EPAD], bf16, kind="ExternalInput").ap()
    projT_d = nc.dram_tensor(
        "projT", [HASH_DIM, MODEL_DIM], bf16, kind="ExternalInput"
    ).ap()
    ident_d = nc.dram_tensor("ident", [P, P], bf16, kind="ExternalInput").ap()
    out_d = nc.dram_tensor("out", [P * K, MODEL_DIM], bf16, kind="ExternalOutput").ap()

    # block -> (chunk, sub-block) map; chunk col offsets into idx16
    blk_ch, blk_sub, col_off = [], [], []
    off = 0
    for ch, nt in enumerate(nts):
        col_off.append(off)
        off += nt // 16
        for s in range(nt // P):
            blk_ch.append(ch)
            blk_sub.append(s)
    assert len(blk_ch) == K

    with tile.TileContext(nc) as tc, ExitStack() as ctx:
        const_p = ctx.enter_context(tc.tile_pool(name="const", bufs=1))
        idx_p = ctx.enter_context(tc.tile_pool(name="idx", bufs=1))
        emb_p = ctx.enter_context(tc.tile_pool(name="emb", bufs=6))
        out_p = ctx.enter_context(tc.tile_pool(name="out", bufs=8))
        # transpose=True gather: no PE transposes, no eT copies, no ps_t
        # pool — all 8 PSUM banks hold 1-bank matmul tiles.
        ps_mm = ctx.enter_context(tc.tile_pool(name="ps_mm", bufs=8, space="PSUM"))

        # idx first: the gather stream depends only on it. Load the first
        # chunk's columns separately so gather 0 is ungated fast.
        idx_t = idx_p.tile([P, ncols], mybir.dt.int16)
        f_hi = nts[0] // 16
        nc.sync.dma_start(out=idx_t[:, :f_hi], in_=idx_d[:, :f_hi])
        if f_hi < ncols:
            nc.sync.dma_start(out=idx_t[:, f_hi:], in_=idx_d[:, f_hi:])
        # rhs rows 64-127 multiply the table's zero-pad dims — value is
        # irrelevant; duplicating projT there keeps the load simple.
        projT_s = const_p.tile([P, MODEL_DIM], bf16)
        nc.sync.dma_start(out=projT_s[:HASH_DIM, :], in_=projT_d[:])
        nc.sync.dma_start(out=projT_s[HASH_DIM:, :], in_=projT_d[:])
        # PE warm-up during the DMA-wait ramp: releases the HAM clock gate
        # before the first real matmul.
        dumw = const_p.tile([P, NFREE], bf16)
        nc.vector.memset(dumw[:], 0.0)
        wps = ps_mm.tile([P, NFREE], f32, name="mm")
        for _ in range(8):
            nc.tensor.matmul(
                wps[:], lhsT=dumw[:, :P], rhs=dumw[:], start=True, stop=True
            )

        emb_tiles = {}
        for ch in range(NCH):
            nt = nts[ch]
            embT = emb_p.tile([P, nt], bf16, name="embT")
            embT3 = embT[:].rearrange("p (c t) -> p c t", c=1)
            nc.gpsimd.dma_gather(
                embT3,
                tab_d[bases[ch] : bases[ch] + W, :],
                idx_t[:, col_off[ch] : col_off[ch] + nt // 16],
                nt,
                nt,
                EPAD,
                transpose=True,
                single_packet=False,
            )
            emb_tiles[ch] = embT

        for b in range(K):
            embT = emb_tiles[blk_ch[b]]
            lhsT = embT[:, blk_sub[b] * P : (blk_sub[b] + 1) * P]
            o_t = out_p.tile([P, MODEL_DIM], bf16, name="o_t")
            for h in range(MODEL_DIM // NFREE):
                mm = ps_mm.tile([P, NFREE], f32, name="mm")
                nc.tensor.matmul(
                    mm[:],
                    lhsT=lhsT,
                    rhs=projT_s[:, h * NFREE : (h + 1) * NFREE],
                    start=True,
                    stop=True,
                )
                dst = o_t[:, h * NFREE : (h + 1) * NFREE]
                if (b + h) % 2 == 0:
                    nc.vector.tensor_copy(dst, mm[:])
                else:
                    nc.scalar.copy(dst, mm[:])
            nc.sync.dma_start(out=out_d[b * P : (b + 1) * P, :], in_=o_t[:])
    nc.compile()
    return nc


def _build_ind_program(K: int) -> "bacc.Bacc":
    """Fallback: per-128-token-block indirect DMA gather (int32 ids).

    HW semantics allow only ONE offset per partition per call, so this path
    costs ~1.4us of gpsimd per 128 tokens — correct for any input, slow."""
    nc = bacc.Bacc(
        "TRN2",
        target_bir_lowering=False,
        debug=False,
        num_devices=N_CORES,
        dynamic_dma_scratch_size=65536,
    )
    f32 = mybir.dt.float32
    bf16 = mybir.dt.bfloat16
    idx_d = nc.dram_tensor("idx", [P, K], mybir.dt.int32, kind="ExternalInput").ap()
    tab_d = nc.dram_tensor("table", [SHARD, HASH_DIM], bf16, kind="ExternalInput").ap()
    projT_d = nc.dram_tensor(
        "projT", [HASH_DIM, MODEL_DIM], bf16, kind="ExternalInput"
    ).ap()
    ident_d = nc.dram_tensor("ident", [P, P], bf16, kind="ExternalInput").ap()
    out_d = nc.dram_tensor("out", [P * K, MODEL_DIM], bf16, kind="ExternalOutput").ap()

    with tile.TileContext(nc) as tc, ExitStack() as ctx:
        const_p = ctx.enter_context(tc.tile_pool(name="const", bufs=1))
        idx_p = ctx.enter_context(tc.tile_pool(name="idx", bufs=1))
        emb_p = ctx.enter_context(tc.tile_pool(name="emb", bufs=6))
        embT_p = ctx.enter_context(tc.tile_pool(name="embT", bufs=4))
        out_p = ctx.enter_context(tc.tile_pool(name="out", bufs=6))
        ps_t = ctx.enter_context(tc.tile_pool(name="ps_t", bufs=2, space="PSUM"))
        ps_mm = ctx.enter_context(tc.tile_pool(name="ps_mm", bufs=3, space="PSUM"))

        idx_t = idx_p.tile([P, K], mybir.dt.int32)
        nc.sync.dma_start(out=idx_t[:], in_=idx_d[:])
        ident = const_p.tile([P, P], bf16)
        nc.sync.dma_start(out=ident[:], in_=ident_d[:])
        projT_s = const_p.tile([P, MODEL_DIM], bf16)
        nc.sync.dma_start(out=projT_s[:HASH_DIM, :], in_=projT_d[:])
        nc.sync.dma_start(out=projT_s[HASH_DIM:, :], in_=projT_d[:])

        pbs = list(range(0, K, 2))
        if K % 2:
            pbs = pbs[-1:] + pbs[:-1]
        cast_i = 0
        for pb in pbs:
            nblocks = min(2, K - pb)
            embp = emb_p.tile([P, nblocks * HASH_DIM], bf16)
            for j in range(nblocks):
                nc.gpsimd.indirect_dma_start(
                    out=embp[:, j * HASH_DIM : (j + 1) * HASH_DIM],
                    out_offset=None,
                    in_=tab_d[:],
                    in_offset=IndirectOffsetOnAxis(
                        ap=idx_t[:, pb + j : pb + j + 1], axis=0
                    ),
                )
            eT_ps = ps_t.tile([nblocks * HASH_DIM, P], bf16)
            nc.tensor.transpose(eT_ps[:], embp[:], ident[:])
            eT = embT_p.tile([nblocks * HASH_DIM, P], bf16)
            if cast_i % 2 == 0:
                nc.vector.tensor_copy(eT[:], eT_ps[:])
            else:
                nc.scalar.copy(eT[:], eT_ps[:])
            mms = [ps_mm.tile([P, MODEL_DIM], f32, name="mm") for _ in range(nblocks)]
            for h in range(MODEL_DIM // NFREE):
                for jj in range(nblocks):
                    nc.tensor.matmul(
                        mms[jj][:, h * NFREE : (h + 1) * NFREE],
                        lhsT=eT[jj * HASH_DIM : (jj + 1) * HASH_DIM, :],
                        rhs=projT_s[
                            jj * HASH_DIM : (jj + 1) * HASH_DIM,
                            h * NFREE : (h + 1) * NFREE,
                        ],
                        start=True,
                        stop=True,
                    )
            for jj in range(nblocks):
                o_t = out_p.tile([P, MODEL_DIM], bf16, name="o_t")
                if (cast_i + jj) % 2 == 0:
                    nc.vector.tensor_copy(o_t[:], mms[jj][:])
                else:
                    nc.scalar.copy(o_t[:], mms[jj][:])
                nc.sync.dma_start(
                    out=out_d[(pb + jj) * P : (pb + jj + 1) * P, :], in_=o_t[:]
                )
            cast_i += 1
    nc.compile()
    return nc


def prepare(input_ids, table, proj_w):
    """Route tokens, pick program variant, build per-core in_maps."""
    B, S = input_ids.shape
    T = B * S
    ids = np.asarray(input_ids, dtype=np.int64)
    prev = np.empty_like(ids)
    prev[:, 0] = 0
    prev[:, 1:] = ids[:, :-1]
    h = ((prev * HASH_MULT + ids) % NUM_BUCKETS).reshape(-1)
    owner = h // SHARD
    local = (h - owner * SHARD).astype(np.int64)
    order = np.lexsort((local, owner))
    counts = np.bincount(owner, minlength=N_CORES).astype(np.int64)
    offsets = np.zeros(N_CORES + 1, dtype=np.int64)
    np.cumsum(counts, out=offsets[1:])
    sorted_local = local[order]

    cap = max(P, int(-(-counts.max() // P)) * P)
    K = cap // P

    # Spread each core's real ids evenly over the cap slots; pads forward-
    # fill so the padded sequence stays sorted and chunk windows stay tight.
    pos_list, padded_list = [], []
    for c in range(N_CORES):
        loc = sorted_local[offsets[c] : offsets[c + 1]]
        n = len(loc)
        if n == 0:
            pos = np.zeros(0, dtype=np.int64)
            row = np.zeros(cap, dtype=np.int64)
        else:
            pos = (np.arange(n, dtype=np.int64) * cap) // n
            row = np.zeros(cap, dtype=np.int64)
            row[pos] = loc
            mark = np.full(cap, -1, dtype=np.int64)
            mark[pos] = np.arange(cap, dtype=np.int64)[pos]
            np.maximum.accumulate(mark, out=mark)
            row = row[np.maximum(mark, 0)]
        pos_list.append(pos)
        padded_list.append(row)
    padded_all = np.stack(padded_list)  # [N_CORES, cap]

    # Greedy variable chunks (multiples of 128 tokens, <= NT) with exact
    # cross-core window bases. Chunk 0 is a single block so the odd/solo
    # block runs in the PE ramp and every later block pairs up.
    lo_all = padded_all.min(axis=0)
    hi_all = padded_all.max(axis=0)
    nts, bases = [], []
    ok = VARIANT == "dg"
    start = 0
    while start < cap and ok:
        b = min(max(int(lo_all[start]), 0), SHARD - W)
        limit = P if start == 0 else min(NT, cap - start)
        size = 0
        for step in range(P, limit + P, P):
            if start + step > cap:
                break
            if int(hi_all[start + step - 1]) - b <= W - 1:
                size = step
            else:
                break
        if size == 0:
            ok = False
            break
        nts.append(size)
        bases.append(b)
        start += size
    if not ok:
        nts, bases = [cap], [0]
    NCH = len(nts)
    cuts = np.zeros(NCH + 1, dtype=np.int64)
    np.cumsum(nts, out=cuts[1:])
    bases = tuple(bases)
    nts = tuple(nts)

    table = np.asarray(table, dtype=np.float32)
    projT = np.ascontiguousarray(
        np.asarray(proj_w, dtype=np.float32).T.astype(ml_dtypes.bfloat16)
    )
    in_maps = []
    for c in range(N_CORES):
        lo, hi = c * SHARD, min((c + 1) * SHARD, NUM_BUCKETS)
        ncols_tab = EPAD if ok else HASH_DIM
        shard = np.zeros((SHARD, ncols_tab), dtype=ml_dtypes.bfloat16)
        shard[: hi - lo, :HASH_DIM] = table[lo:hi].astype(ml_dtypes.bfloat16)
        m = {"table": shard, "projT": projT}
        if ok:
            # idx16[p, col_off+s] = chunk token s*16+p, relative to the
            # chunk base; wrapped in 16 partitions, replicated to all 8
            # 16-partition Q7 core groups.
            rel = padded_all[c].copy()
            for ch in range(NCH):
                rel[cuts[ch] : cuts[ch + 1]] -= bases[ch]
            rel = np.maximum(rel, 0)
            cols = [
                rel[cuts[ch] : cuts[ch + 1]].reshape(-1, 16).T for ch in range(NCH)
            ]
            row16 = np.concatenate(cols, axis=1).astype(np.int16)
            m["idx16"] = np.ascontiguousarray(np.tile(row16, (P // 16, 1)))
        else:
            padded = np.zeros(cap, dtype=np.int64)
            padded[: counts[c]] = sorted_local[offsets[c] : offsets[c + 1]]
            m["idx"] = np.ascontiguousarray(padded.astype(np.int32).reshape(K, P).T)
        m["ident"] = np.eye(P, dtype=ml_dtypes.bfloat16)
        in_maps.append(m)

    key = ("dg", K, nts, bases) if ok else ("ind", K)
    nc = _prog_cache.get(key)
    if nc is None:
        nc = _build_dg_program(K, bases, nts) if ok else _build_ind_program(K)
        _prog_cache[key] = nc
    # row_map[c]: device out row holding sorted token k of core c
    if ok:
        row_map = pos_list
    else:
        row_map = [np.arange(counts[c], dtype=np.int64) for c in range(N_CORES)]
    meta = (T, order, offsets, row_map, K)
    return nc, in_maps, meta


def kernel(input_ids: np.ndarray, table: np.ndarray, proj_w: np.ndarray) -> np.ndarray:
    B, S = input_ids.shape
    nc, in_maps, meta = prepare(input_ids, table, proj_w)
    T, order, offsets, row_map, K = meta
    res = run_bass_kernel_spmd(nc, in_maps, list(range(N_CORES)))
    flat = np.empty((T, MODEL_DIM), dtype=np.float32)
    for c in range(N_CORES):
        flat[order[offsets[c] : offsets[c + 1]]] = res.results[c]["out"][
            row_map[c]
        ].astype(np.float32)
    return flat.reshape(B, S, MODEL_DIM)
